# revision 1
# baseline (speedup 1.0000x reference)
"""InnerAttention kernel for 8 Trainium2 NeuronCores.

Computes, per batch b:
    e = x[b] @ y[b].T          [M, N]
    p = softmax(e, axis=-1)    (over n)
    out[b] = p.T @ x[b]        [N, D]

Sharding: data-parallel over batch (B=8 -> one batch per core). Full inputs in,
full output out; per-core slices are shipped via run_bass_kernel_spmd in_maps.

Per-core algorithm (M=N=2048, D=1024, P=128):
  prep:  y -> yTh/yTl (bf16 hi/lo, d-major, per-n-slice tiles) via PE transposes
  loop over 16 m-tiles (groups of 4):
    x m-tile -> xTh/xTl (bf16 hi/lo, d-major) via PE transposes
    mm1: e[128, 2048] in PSUM = xh@yh + xh@yl + xl@yh   (3-pass bf16 split,
         fp32-grade: representation error ~2^-18/elem), weight-stationary order
    softmax: DVE row-max over PSUM, ACT exp (bias=-max) -> p (fp32r, rounded),
         accum_out gives row-sum for free; 1/sum folded into x' = x * (1/s)
    mm2 (per group of 4 m-tiles): out_psum[n-chunk 128, d 512] accumulates
         p.T @ x' over the group (fp32r 1-pass), then DVE-stage + SWDGE DMA
         with accum_op=add flushes into DRAM out.
"""

import numpy as np

import concourse.bacc as bacc
import concourse.mybir as mybir
import concourse.tile as tile
from concourse import bass_utils

B, M, N, D = 8, 2048, 2048, 1024
P = 128
NSLICE = 512          # matmul moving free-dim (one PSUM bank of fp32)
N_MTILES = M // P     # 16
N_DCHUNK = D // P     # 8
N_NSL = N // NSLICE   # 4
GROUP = 8             # m-tiles per mm2 accumulation group
N_GROUPS = N_MTILES // GROUP
N_NCHUNK = N // P     # 16
N_DHALF = D // NSLICE  # 2

F32 = mybir.dt.float32
F32R = mybir.dt.float32r
BF16 = mybir.dt.bfloat16
AX = mybir.AxisListType.X
EXP = mybir.ActivationFunctionType.Exp
ADD = mybir.AluOpType.add


def _build_nc():
    nc = bacc.Bacc("TRN2", target_bir_lowering=False, debug=False)
    x_d = nc.dram_tensor("x", [M, D], F32, kind="ExternalInput").ap()
    y_d = nc.dram_tensor("y", [N, D], F32, kind="ExternalInput").ap()
    ident_d = nc.dram_tensor("ident", [P, P], F32, kind="ExternalInput").ap()
    out_d = nc.dram_tensor("out", [N, D], F32, kind="ExternalOutput").ap()

    with tile.TileContext(nc) as tc:
        with (
            tc.tile_pool(name="const", bufs=1) as constp,
            tc.tile_pool(name="yT", bufs=1) as yTp,
            tc.tile_pool(name="pg", bufs=1) as pgp,
            tc.tile_pool(name="xg", bufs=1) as xgp,
            tc.tile_pool(name="work", bufs=2) as work,
            tc.tile_pool(name="stats", bufs=3) as stats,
            tc.tile_pool(name="eps", bufs=4, space="PSUM") as epsp,
            tc.tile_pool(name="tps", bufs=2, space="PSUM") as tpsp,
            tc.tile_pool(name="ops", bufs=2, space="PSUM") as opsp,
        ):
            # identity for PE transposes (f32)
            ident = constp.tile([P, P], F32, tag="id32")
            nc.sync.dma_start(ident[:], ident_d)

            # persistent yT hi/lo: per n-slice tiles [128 d, 8 chunks x 512 n]
            # bf16; chunk k occupies columns [k*512, (k+1)*512)
            yTh = [yTp.tile([P, N_DCHUNK * NSLICE], BF16, tag=f"yTh{s}", name=f"yTh{s}")
                   for s in range(N_NSL)]
            yTl = [yTp.tile([P, N_DCHUNK * NSLICE], BF16, tag=f"yTl{s}", name=f"yTl{s}")
                   for s in range(N_NSL)]

            def transpose_split(src_f32, sink):
                """PE-transpose f32 [128, 1024] in two 4-chunk batches; sink
                consumes (half, psum [128, 4x128] f32) and derives bf16 hi/lo."""
                for half in range(2):
                    ps = tpsp.tile([P, NSLICE], F32, tag="tp", name="tp")
                    for j in range(4):
                        k = half * 4 + j
                        nc.tensor.transpose(
                            ps[:, j * P:(j + 1) * P],
                            src_f32[:, k * P:(k + 1) * P],
                            ident[:],
                        )
                    sink(half, ps)

            # ---- prep: y -> yTh/yTl ----
            for i in range(N // P):
                sl, c0 = i // 4, (i % 4) * P
                ynat = work.tile([P, D], F32, tag="ynat")
                nc.sync.dma_start(ynat[:], y_d[i * P:(i + 1) * P, :])

                def ysink(half, ps, sl=sl, c0=c0):
                    # dst AP: chunks half*4..half*4+4, 128 cols at offset c0
                    hi = yTh[sl].rearrange("p (k c) -> p k c", c=NSLICE)[
                        :, half * 4:half * 4 + 4, c0:c0 + P]
                    lo = yTl[sl].rearrange("p (k c) -> p k c", c=NSLICE)[
                        :, half * 4:half * 4 + 4, c0:c0 + P]
                    src = ps.rearrange("p (j c) -> p j c", c=P)
                    nc.vector.tensor_copy(hi, src)
                    nc.vector.tensor_sub(lo, src, hi)
                transpose_split(ynat, ysink)

            # ---- main loop ----
            for g in range(N_GROUPS):
                pg = []
                xg = []
                for mi in range(GROUP):
                    m = g * GROUP + mi
                    xnat = work.tile([P, D], F32, tag="xnat")
                    nc.sync.dma_start(xnat[:], x_d[m * P:(m + 1) * P, :])
                    xTh = work.tile([P, D], BF16, tag="xTh", name="xTh")
                    xTl = work.tile([P, D], BF16, tag="xTl", name="xTl")

                    def xsink(half, ps):
                        c0 = half * NSLICE
                        nc.vector.tensor_copy(xTh[:, c0:c0 + NSLICE], ps[:])
                        nc.vector.tensor_sub(
                            xTl[:, c0:c0 + NSLICE], ps[:], xTh[:, c0:c0 + NSLICE])
                    transpose_split(xnat, xsink)

                    # mm1: e block [128, 2048] across 4 psum slices,
                    # weight-stationary (lhsT constant across the ns loop)
                    eps = [epsp.tile([P, NSLICE], F32, tag="e", name="eps")
                           for _ in range(N_NSL)]
                    for ns in range(N_NSL):
                        for k in range(N_DCHUNK):
                            nc.tensor.matmul(
                                eps[ns][:], xTh[:, k * P:(k + 1) * P],
                                yTh[ns][:, k * NSLICE:(k + 1) * NSLICE],
                                start=(k == 0), stop=False,
                            )
                        for k in range(N_DCHUNK):
                            nc.tensor.matmul(
                                eps[ns][:], xTh[:, k * P:(k + 1) * P],
                                yTl[ns][:, k * NSLICE:(k + 1) * NSLICE],
                                start=False, stop=False,
                            )
                        for k in range(N_DCHUNK):
                            nc.tensor.matmul(
                                eps[ns][:], xTl[:, k * P:(k + 1) * P],
                                yTh[ns][:, k * NSLICE:(k + 1) * NSLICE],
                                start=False, stop=(k == N_DCHUNK - 1),
                            )

                    # softmax over the row of 2048
                    rmax4 = stats.tile([P, N_NSL], F32, tag="rmax4")
                    for ns in range(N_NSL):
                        nc.vector.reduce_max(rmax4[:, ns:ns + 1], eps[ns][:], axis=AX)
                    negmax = stats.tile([P, 1], F32, tag="negmax")
                    rmax = stats.tile([P, 1], F32, tag="rmax")
                    nc.vector.reduce_max(rmax[:], rmax4[:], axis=AX)
                    nc.vector.tensor_scalar_mul(negmax[:], rmax[:], -1.0)

                    ph = pgp.tile([P, N], F32R, tag=f"pg{mi}", name=f"pg{mi}")
                    s4 = stats.tile([P, N_NSL], F32, tag="s4")
                    for ns in range(N_NSL):
                        c0 = ns * NSLICE
                        nc.scalar.activation(
                            ph[:, c0:c0 + NSLICE], eps[ns][:], EXP,
                            bias=negmax[:], accum_out=s4[:, ns:ns + 1],
                        )
                    ssum = stats.tile([P, 1], F32, tag="ssum")
                    nc.vector.reduce_sum(ssum[:], s4[:], axis=AX)
                    rinv = stats.tile([P, 1], F32, tag="rinv")
                    nc.vector.reciprocal(rinv[:], ssum[:])
                    xs = xgp.tile([P, D], F32R, tag=f"xg{mi}", name=f"xg{mi}")
                    nc.vector.tensor_scalar_mul(xs[:], xnat[:], rinv[:])
                    pg.append(ph)
                    xg.append(xs)

                # mm2 for this group: out[nchunk, dhalf] += p.T @ x'
                for nch in range(N_NCHUNK):
                    for dh in range(N_DHALF):
                        ops = opsp.tile([P, NSLICE], F32, tag="o", name="ops")
                        for mi in range(GROUP):
                            nc.tensor.matmul(
                                ops[:],
                                pg[mi][:, nch * P:(nch + 1) * P],
                                xg[mi][:, dh * NSLICE:(dh + 1) * NSLICE],
                                start=(mi == 0), stop=(mi == GROUP - 1),
                            )
                        dst = out_d[nch * P:(nch + 1) * P,
                                    dh * NSLICE:(dh + 1) * NSLICE]
                        stage = work.tile([P, NSLICE], F32, tag="ostage", bufs=4)
                        nc.vector.tensor_copy(stage[:], ops[:])
                        if g == 0:
                            nc.gpsimd.dma_start(dst, stage[:])
                        else:
                            nc.gpsimd.dma_start(dst, stage[:], accum_op=ADD)

    nc.compile()
    return nc


_NC_CACHE = {}


def _get_nc():
    if "nc" not in _NC_CACHE:
        _NC_CACHE["nc"] = _build_nc()
    return _NC_CACHE["nc"]


def kernel(x: np.ndarray, y: np.ndarray) -> np.ndarray:
    assert x.shape == (B, M, D) and y.shape == (B, N, D)
    nc = _get_nc()
    ident = np.eye(P, dtype=np.float32)
    in_maps = [
        {
            "x": np.ascontiguousarray(x[b], dtype=np.float32),
            "y": np.ascontiguousarray(y[b], dtype=np.float32),
            "ident": ident,
        }
        for b in range(B)
    ]
    res = bass_utils.run_bass_kernel_spmd(nc, in_maps, core_ids=list(range(B)))
    return np.stack([res.results[b]["out"] for b in range(B)], axis=0)



# revision 4
# speedup vs baseline: 1.5978x; 1.5978x over previous
"""InnerAttention kernel for 8 Trainium2 NeuronCores.

Computes, per batch b:
    e = x[b] @ y[b].T          [M, N]
    p = softmax(e, axis=-1)    (over n)
    out[b] = p.T @ x[b]        [N, D]

Sharding: data-parallel over batch (B=8 -> one batch per core). Full inputs in,
full output out; per-core slices are shipped via run_bass_kernel_spmd in_maps.

Per-core algorithm (M=N=2048, D=1024, P=128):
  prep:  y -> yTh/yTl (bf16 hi/lo, d-major, per-n-slice tiles) via PE transposes
  loop over 16 m-tiles (groups of 4):
    x m-tile -> xTh/xTl (bf16 hi/lo, d-major) via PE transposes
    mm1: e[128, 2048] in PSUM = xh@yh + xh@yl + xl@yh   (3-pass bf16 split,
         fp32-grade: representation error ~2^-18/elem), weight-stationary order
    softmax: DVE row-max over PSUM, ACT exp (bias=-max) -> p (fp32r, rounded),
         accum_out gives row-sum for free; 1/sum folded into x' = x * (1/s)
    mm2 (per group of 4 m-tiles): out_psum[n-chunk 128, d 512] accumulates
         p.T @ x' over the group (fp32r 1-pass), then DVE-stage + SWDGE DMA
         with accum_op=add flushes into DRAM out.
"""

import numpy as np

import concourse.bacc as bacc
import concourse.mybir as mybir
import concourse.tile as tile
from concourse import bass_utils

B, M, N, D = 8, 2048, 2048, 1024
P = 128
NSLICE = 512          # matmul moving free-dim (one PSUM bank of fp32)
N_MTILES = M // P     # 16
N_DCHUNK = D // P     # 8
N_NSL = N // NSLICE   # 4
GROUP = 8             # m-tiles per mm2 accumulation group
N_GROUPS = N_MTILES // GROUP
N_NCHUNK = N // P     # 16
N_DHALF = D // NSLICE  # 2

F32 = mybir.dt.float32
F32R = mybir.dt.float32r
BF16 = mybir.dt.bfloat16
AX = mybir.AxisListType.X
EXP = mybir.ActivationFunctionType.Exp
ADD = mybir.AluOpType.add


def _build_nc():
    nc = bacc.Bacc("TRN2", target_bir_lowering=False, debug=False)
    x_d = nc.dram_tensor("x", [M, D], F32, kind="ExternalInput").ap()
    y_d = nc.dram_tensor("y", [N, D], F32, kind="ExternalInput").ap()
    ident_d = nc.dram_tensor("ident", [P, P], F32, kind="ExternalInput").ap()
    out_d = nc.dram_tensor("out", [N, D], F32, kind="ExternalOutput").ap()

    with tile.TileContext(nc) as tc:
        with (
            tc.tile_pool(name="const", bufs=1) as constp,
            tc.tile_pool(name="yT", bufs=1) as yTp,
            tc.tile_pool(name="pg", bufs=1) as pgp,
            tc.tile_pool(name="xg", bufs=1) as xgp,
            tc.tile_pool(name="work", bufs=2) as work,
            tc.tile_pool(name="stats", bufs=3) as stats,
            tc.tile_pool(name="eps", bufs=4, space="PSUM") as epsp,
            tc.tile_pool(name="tps", bufs=2, space="PSUM") as tpsp,
            tc.tile_pool(name="ops", bufs=2, space="PSUM") as opsp,
        ):
            # identity for PE transposes (f32)
            ident = constp.tile([P, P], F32, tag="id32")
            nc.sync.dma_start(ident[:], ident_d)

            # persistent yT: per n-slice tiles [128 d, 8 chunks x 512 n]
            # fp32r; chunk k occupies columns [k*512, (k+1)*512)
            yT = [yTp.tile([P, N_DCHUNK * NSLICE], F32R, tag=f"yT{s}", name=f"yT{s}")
                  for s in range(N_NSL)]

            def transpose_split(src_f32, sink):
                """PE-transpose f32 [128, 1024] in two 4-chunk batches; sink
                consumes (half, psum [128, 4x128] f32) and derives bf16 hi/lo."""
                for half in range(2):
                    ps = tpsp.tile([P, NSLICE], F32, tag="tp", name="tp")
                    for j in range(4):
                        k = half * 4 + j
                        nc.tensor.transpose(
                            ps[:, j * P:(j + 1) * P],
                            src_f32[:, k * P:(k + 1) * P],
                            ident[:],
                        )
                    sink(half, ps)

            # ---- prep: y -> yTh/yTl ----
            for i in range(N // P):
                sl, c0 = i // 4, (i % 4) * P
                ynat = work.tile([P, D], F32, tag="ynat")
                nc.sync.dma_start(ynat[:], y_d[i * P:(i + 1) * P, :])

                def ysink(half, ps, sl=sl, c0=c0):
                    # dst AP: chunks half*4..half*4+4, 128 cols at offset c0
                    dst = yT[sl].rearrange("p (k c) -> p k c", c=NSLICE)[
                        :, half * 4:half * 4 + 4, c0:c0 + P]
                    src = ps.rearrange("p (j c) -> p j c", c=P)
                    nc.vector.tensor_copy(dst, src)
                transpose_split(ynat, ysink)

            # ---- main loop ----
            for g in range(N_GROUPS):
                pg = []
                xg = []
                for mi in range(GROUP):
                    m = g * GROUP + mi
                    xnat = work.tile([P, D], F32, tag="xnat")
                    nc.sync.dma_start(xnat[:], x_d[m * P:(m + 1) * P, :])
                    xT = work.tile([P, D], F32R, tag="xT", name="xT")

                    def xsink(half, ps):
                        c0 = half * NSLICE
                        nc.vector.tensor_copy(xT[:, c0:c0 + NSLICE], ps[:])
                    transpose_split(xnat, xsink)

                    # mm1: e block [128, 2048] across 4 psum slices,
                    # fp32r single pass, weight-stationary
                    eps = [epsp.tile([P, NSLICE], F32, tag="e", name="eps")
                           for _ in range(N_NSL)]
                    for ns in range(N_NSL):
                        for k in range(N_DCHUNK):
                            nc.tensor.matmul(
                                eps[ns][:], xT[:, k * P:(k + 1) * P],
                                yT[ns][:, k * NSLICE:(k + 1) * NSLICE],
                                start=(k == 0), stop=(k == N_DCHUNK - 1),
                            )

                    # softmax over the row of 2048
                    rmax4 = stats.tile([P, N_NSL], F32, tag="rmax4")
                    for ns in range(N_NSL):
                        nc.vector.reduce_max(rmax4[:, ns:ns + 1], eps[ns][:], axis=AX)
                    negmax = stats.tile([P, 1], F32, tag="negmax")
                    rmax = stats.tile([P, 1], F32, tag="rmax")
                    nc.vector.reduce_max(rmax[:], rmax4[:], axis=AX)
                    nc.vector.tensor_scalar_mul(negmax[:], rmax[:], -1.0)

                    ph = pgp.tile([P, N], F32R, tag=f"pg{mi}", name=f"pg{mi}")
                    s4 = stats.tile([P, N_NSL], F32, tag="s4")
                    for ns in range(N_NSL):
                        c0 = ns * NSLICE
                        nc.scalar.activation(
                            ph[:, c0:c0 + NSLICE], eps[ns][:], EXP,
                            bias=negmax[:], accum_out=s4[:, ns:ns + 1],
                        )
                    ssum = stats.tile([P, 1], F32, tag="ssum")
                    nc.vector.reduce_sum(ssum[:], s4[:], axis=AX)
                    rinv = stats.tile([P, 1], F32, tag="rinv")
                    nc.vector.reciprocal(rinv[:], ssum[:])
                    xs = xgp.tile([P, D], F32R, tag=f"xg{mi}", name=f"xg{mi}")
                    nc.vector.tensor_scalar_mul(xs[:], xnat[:], rinv[:])
                    pg.append(ph)
                    xg.append(xs)

                # mm2 for this group: out[nchunk, dhalf] += p.T @ x'
                for nch in range(N_NCHUNK):
                    for dh in range(N_DHALF):
                        ops = opsp.tile([P, NSLICE], F32, tag="o", name="ops")
                        for mi in range(GROUP):
                            nc.tensor.matmul(
                                ops[:],
                                pg[mi][:, nch * P:(nch + 1) * P],
                                xg[mi][:, dh * NSLICE:(dh + 1) * NSLICE],
                                start=(mi == 0), stop=(mi == GROUP - 1),
                            )
                        dst = out_d[nch * P:(nch + 1) * P,
                                    dh * NSLICE:(dh + 1) * NSLICE]
                        stage = work.tile([P, NSLICE], F32, tag="ostage", bufs=4)
                        nc.vector.tensor_copy(stage[:], ops[:])
                        if g == 0:
                            nc.gpsimd.dma_start(dst, stage[:])
                        else:
                            nc.gpsimd.dma_start(dst, stage[:], accum_op=ADD)

    nc.compile()
    return nc


_NC_CACHE = {}


def _get_nc():
    if "nc" not in _NC_CACHE:
        _NC_CACHE["nc"] = _build_nc()
    return _NC_CACHE["nc"]


def kernel(x: np.ndarray, y: np.ndarray) -> np.ndarray:
    assert x.shape == (B, M, D) and y.shape == (B, N, D)
    nc = _get_nc()
    ident = np.eye(P, dtype=np.float32)
    in_maps = [
        {
            "x": np.ascontiguousarray(x[b], dtype=np.float32),
            "y": np.ascontiguousarray(y[b], dtype=np.float32),
            "ident": ident,
        }
        for b in range(B)
    ]
    res = bass_utils.run_bass_kernel_spmd(nc, in_maps, core_ids=list(range(B)))
    return np.stack([res.results[b]["out"] for b in range(B)], axis=0)



# revision 5
# speedup vs baseline: 1.6863x; 1.0554x over previous
"""InnerAttention kernel for 8 Trainium2 NeuronCores.

Computes, per batch b:
    e = x[b] @ y[b].T          [M, N]
    p = softmax(e, axis=-1)    (over n)
    out[b] = p.T @ x[b]        [N, D]

Sharding: data-parallel over batch (B=8 -> one batch per core). Full inputs in,
full output out; per-core slices are shipped via run_bass_kernel_spmd in_maps.

Per-core algorithm (M=N=2048, D=1024, P=128):
  phase 0: y -> yT (fp32r, d-major, per-n-slice tiles) via PE transposes
  phase 1: per m-tile (16):
    x m-tile -> xT (fp32r, d-major) via PE transposes
    mm1: e[128, 2048] in PSUM, single fp32r pass (1 cycle/row at free=512)
    softmax: DVE row-max over PSUM, ACT exp (bias=-max) -> p bf16 in SBUF,
         accum_out gives row-sum; 1/sum folded into xs = x * (1/s) in bf16
  phase 2: per (n-chunk 128, d-half 512) out tile: accumulate all 16
    p.T @ xs contributions (bf16) in one PSUM bank, stage, single DMA out.
"""

import numpy as np

import concourse.bacc as bacc
import concourse.mybir as mybir
import concourse.tile as tile
from concourse import bass_utils

B, M, N, D = 8, 2048, 2048, 1024
P = 128
NSLICE = 512          # matmul moving free-dim (one PSUM bank of fp32)
N_MTILES = M // P     # 16
N_DCHUNK = D // P     # 8
N_NSL = N // NSLICE   # 4
N_NCHUNK = N // P     # 16
N_DHALF = D // NSLICE  # 2

F32 = mybir.dt.float32
F32R = mybir.dt.float32r
BF16 = mybir.dt.bfloat16
AX = mybir.AxisListType.X
EXP = mybir.ActivationFunctionType.Exp


def _build_nc():
    nc = bacc.Bacc("TRN2", target_bir_lowering=False, debug=False)
    x_d = nc.dram_tensor("x", [M, D], F32, kind="ExternalInput").ap()
    y_d = nc.dram_tensor("y", [N, D], F32, kind="ExternalInput").ap()
    ident_d = nc.dram_tensor("ident", [P, P], F32, kind="ExternalInput").ap()
    out_d = nc.dram_tensor("out", [N, D], F32, kind="ExternalOutput").ap()

    with tile.TileContext(nc) as tc:
        with (
            tc.tile_pool(name="const", bufs=1) as constp,
            tc.tile_pool(name="yT", bufs=1) as yTp,
            tc.tile_pool(name="pP", bufs=1) as pPp,
            tc.tile_pool(name="xsP", bufs=1) as xsPp,
            tc.tile_pool(name="work", bufs=2) as work,
            tc.tile_pool(name="stats", bufs=3) as stats,
        ):
            # identity for PE transposes (f32)
            ident = constp.tile([P, P], F32, tag="id32")
            nc.sync.dma_start(ident[:], ident_d)

            # persistent yT: per n-slice tiles [128 d, 8 chunks x 512 n]
            # fp32r; chunk k occupies columns [k*512, (k+1)*512)
            yT = [yTp.tile([P, N_DCHUNK * NSLICE], F32R, tag=f"yT{s}", name=f"yT{s}")
                  for s in range(N_NSL)]
            # persistent p (bf16) and scaled-x (bf16) for mm2
            pT = [pPp.tile([P, N], BF16, tag=f"p{mi}", name=f"p{mi}")
                  for mi in range(N_MTILES)]
            xs = [xsPp.tile([P, D], BF16, tag=f"xs{mi}", name=f"xs{mi}")
                  for mi in range(N_MTILES)]

            with tc.tile_pool(name="ps", bufs=8, space="PSUM") as psp:
                def transpose_split(src_f32, sink):
                    """PE-transpose f32 [128, 1024] in two 4-chunk batches; sink
                    consumes (half, psum [128, 4x128] f32)."""
                    for half in range(2):
                        ps = psp.tile([P, NSLICE], F32, tag="ps", name="tp")
                        for j in range(4):
                            k = half * 4 + j
                            nc.tensor.transpose(
                                ps[:, j * P:(j + 1) * P],
                                src_f32[:, k * P:(k + 1) * P],
                                ident[:],
                            )
                        sink(half, ps)

                # ---- phase 0: y -> yT ----
                for i in range(N // P):
                    sl, c0 = i // 4, (i % 4) * P
                    ynat = work.tile([P, D], F32, tag="ynat")
                    nc.sync.dma_start(ynat[:], y_d[i * P:(i + 1) * P, :])

                    def ysink(half, ps, sl=sl, c0=c0):
                        # dst AP: chunks half*4..half*4+4, 128 cols at offset c0
                        dst = yT[sl].rearrange("p (k c) -> p k c", c=NSLICE)[
                            :, half * 4:half * 4 + 4, c0:c0 + P]
                        src = ps.rearrange("p (j c) -> p j c", c=P)
                        nc.vector.tensor_copy(dst, src)
                    transpose_split(ynat, ysink)

                # ---- phase 1: mm1 + softmax per m-tile ----
                for mi in range(N_MTILES):
                    m = mi * P
                    xnat = work.tile([P, D], F32, tag="xnat")
                    nc.sync.dma_start(xnat[:], x_d[m:m + P, :])
                    xT = work.tile([P, D], F32R, tag="xT", name="xT")

                    def xsink(half, ps):
                        c0 = half * NSLICE
                        nc.vector.tensor_copy(xT[:, c0:c0 + NSLICE], ps[:])
                    transpose_split(xnat, xsink)

                    # mm1: e block [128, 2048] across 4 psum banks, fp32r
                    eps = []
                    for ns in range(N_NSL):
                        ep = psp.tile([P, NSLICE], F32, tag="ps", name="eps")
                        for k in range(N_DCHUNK):
                            nc.tensor.matmul(
                                ep[:], xT[:, k * P:(k + 1) * P],
                                yT[ns][:, k * NSLICE:(k + 1) * NSLICE],
                                start=(k == 0), stop=(k == N_DCHUNK - 1),
                            )
                        eps.append(ep)

                    # softmax over the row of 2048
                    rmax4 = stats.tile([P, N_NSL], F32, tag="rmax4")
                    for ns in range(N_NSL):
                        nc.vector.reduce_max(rmax4[:, ns:ns + 1], eps[ns][:], axis=AX)
                    negmax = stats.tile([P, 1], F32, tag="negmax")
                    rmax = stats.tile([P, 1], F32, tag="rmax")
                    nc.vector.reduce_max(rmax[:], rmax4[:], axis=AX)
                    nc.vector.tensor_scalar_mul(negmax[:], rmax[:], -1.0)

                    s4 = stats.tile([P, N_NSL], F32, tag="s4")
                    for ns in range(N_NSL):
                        c0 = ns * NSLICE
                        nc.scalar.activation(
                            pT[mi][:, c0:c0 + NSLICE], eps[ns][:], EXP,
                            bias=negmax[:], accum_out=s4[:, ns:ns + 1],
                        )
                    ssum = stats.tile([P, 1], F32, tag="ssum")
                    nc.vector.reduce_sum(ssum[:], s4[:], axis=AX)
                    rinv = stats.tile([P, 1], F32, tag="rinv")
                    nc.vector.reciprocal(rinv[:], ssum[:])
                    nc.vector.tensor_scalar_mul(xs[mi][:], xnat[:], rinv[:])

            # ---- phase 2: out[nch, dh] = sum_mi p[mi].T @ xs[mi], one flush ----
            with tc.tile_pool(name="ops", bufs=8, space="PSUM") as opsp:
                for nch in range(N_NCHUNK):
                    for dh in range(N_DHALF):
                        ops = opsp.tile([P, NSLICE], F32, tag="o", name="ops")
                        for mi in range(N_MTILES):
                            nc.tensor.matmul(
                                ops[:],
                                pT[mi][:, nch * P:(nch + 1) * P],
                                xs[mi][:, dh * NSLICE:(dh + 1) * NSLICE],
                                start=(mi == 0), stop=(mi == N_MTILES - 1),
                            )
                        dst = out_d[nch * P:(nch + 1) * P,
                                    dh * NSLICE:(dh + 1) * NSLICE]
                        stage = work.tile([P, NSLICE], F32, tag="ostage", bufs=4)
                        nc.vector.tensor_copy(stage[:], ops[:])
                        nc.sync.dma_start(dst, stage[:])

    nc.compile()
    return nc


_NC_CACHE = {}


def _get_nc():
    if "nc" not in _NC_CACHE:
        _NC_CACHE["nc"] = _build_nc()
    return _NC_CACHE["nc"]


def kernel(x: np.ndarray, y: np.ndarray) -> np.ndarray:
    assert x.shape == (B, M, D) and y.shape == (B, N, D)
    nc = _get_nc()
    ident = np.eye(P, dtype=np.float32)
    in_maps = [
        {
            "x": np.ascontiguousarray(x[b], dtype=np.float32),
            "y": np.ascontiguousarray(y[b], dtype=np.float32),
            "ident": ident,
        }
        for b in range(B)
    ]
    res = bass_utils.run_bass_kernel_spmd(nc, in_maps, core_ids=list(range(B)))
    return np.stack([res.results[b]["out"] for b in range(B)], axis=0)


# revision 8
# speedup vs baseline: 1.8540x; 1.0994x over previous
"""InnerAttention kernel for 8 Trainium2 NeuronCores.

Computes, per batch b:
    e = x[b] @ y[b].T          [M, N]
    p = softmax(e, axis=-1)    (over n)
    out[b] = p.T @ x[b]        [N, D]

Sharding: data-parallel over batch (B=8 -> one batch per core). Full inputs in,
full output out; per-core slices are shipped via run_bass_kernel_spmd in_maps.

Per-core algorithm (M=N=2048, D=1024, P=128):
  phase 0: y -> yT (fp32r, d-major, per-n-slice tiles) via PE transposes
  phase 1: per m-tile (16):
    x m-tile -> xT (fp32r, d-major) via PE transposes
    mm1: e[128, 2048] in PSUM, single fp32r pass (1 cycle/row at free=512)
    softmax: DVE row-max over PSUM, ACT exp (bias=-max) -> p bf16 in SBUF,
         accum_out gives row-sum; 1/sum folded into xs = x * (1/s) in bf16
  phase 2: per (n-chunk 128, d-half 512) out tile: accumulate all 16
    p.T @ xs contributions (bf16) in one PSUM bank, stage, single DMA out.
"""

import numpy as np

import concourse.bacc as bacc
import concourse.mybir as mybir
import concourse.tile as tile
from concourse import bass_utils

B, M, N, D = 8, 2048, 2048, 1024
P = 128
NSLICE = 512          # matmul moving free-dim (one PSUM bank of fp32)
N_MTILES = M // P     # 16
N_DCHUNK = D // P     # 8
N_NSL = N // NSLICE   # 4
N_NCHUNK = N // P     # 16
N_DHALF = D // NSLICE  # 2

F32 = mybir.dt.float32
F32R = mybir.dt.float32r
BF16 = mybir.dt.bfloat16
AX = mybir.AxisListType.X
EXP = mybir.ActivationFunctionType.Exp


def _build_nc():
    nc = bacc.Bacc("TRN2", target_bir_lowering=False, debug=False)
    x_d = nc.dram_tensor("x", [M, D], F32R, kind="ExternalInput").ap()
    y_d = nc.dram_tensor("y", [N, D], F32R, kind="ExternalInput").ap()
    ident_d = nc.dram_tensor("ident", [P, P], F32R, kind="ExternalInput").ap()
    out_d = nc.dram_tensor("out", [N, D], F32, kind="ExternalOutput").ap()

    with tile.TileContext(nc) as tc:
        with (
            tc.tile_pool(name="const", bufs=1) as constp,
            tc.tile_pool(name="yT", bufs=1) as yTp,
            tc.tile_pool(name="pP", bufs=1) as pPp,
            tc.tile_pool(name="xsP", bufs=1) as xsPp,
            tc.tile_pool(name="work", bufs=2) as work,
            tc.tile_pool(name="stats", bufs=3) as stats,
        ):
            # identity for PE transposes (fp32r: 1.5 cycles/row vs 2.0 for f32)
            ident = constp.tile([P, P], F32R, tag="id32")
            nc.sync.dma_start(ident[:], ident_d)

            # persistent yT: per n-slice tiles [128 d, 8 chunks x 512 n]
            # fp32r; chunk k occupies columns [k*512, (k+1)*512)
            yT = [yTp.tile([P, N_DCHUNK * NSLICE], F32R, tag=f"yT{s}", name=f"yT{s}")
                  for s in range(N_NSL)]
            # persistent p (bf16) and scaled-x (bf16) for mm2
            pT = [pPp.tile([P, N], BF16, tag=f"p{mi}", name=f"p{mi}")
                  for mi in range(N_MTILES)]
            xs = [xsPp.tile([P, D], BF16, tag=f"xs{mi}", name=f"xs{mi}")
                  for mi in range(N_MTILES)]

            with (
                tc.tile_pool(name="tps", bufs=2, space="PSUM") as tpsp,
                tc.tile_pool(name="eps", bufs=6, space="PSUM") as epsp,
            ):
                def transpose_split(src, sink):
                    """PE-transpose fp32r [128, 1024] in two 4-chunk batches;
                    sink consumes (half, psum [128, 4x128] fp32r)."""
                    for half in range(2):
                        ps = tpsp.tile([P, NSLICE], F32R, tag="tp", name="tp")
                        for j in range(4):
                            k = half * 4 + j
                            nc.tensor.transpose(
                                ps[:, j * P:(j + 1) * P],
                                src[:, k * P:(k + 1) * P],
                                ident[:],
                            )
                        sink(half, ps)

                # ---- phase 0: y -> yT ----
                for i in range(N // P):
                    sl, c0 = i // 4, (i % 4) * P
                    ynat = work.tile([P, D], F32R, tag="ynat", bufs=4)
                    nc.sync.dma_start(ynat[:], y_d[i * P:(i + 1) * P, :])

                    def ysink(half, ps, sl=sl, c0=c0):
                        # dst AP: chunks half*4..half*4+4, 128 cols at offset c0
                        dst = yT[sl].rearrange("p (k c) -> p k c", c=NSLICE)[
                            :, half * 4:half * 4 + 4, c0:c0 + P]
                        src = ps.rearrange("p (j c) -> p j c", c=P)
                        nc.vector.tensor_copy(dst, src)
                    transpose_split(ynat, ysink)

                # ---- phase 1: mm1 + softmax per m-tile ----
                for mi in range(N_MTILES):
                    m = mi * P
                    xnat = work.tile([P, D], F32R, tag="xnat", bufs=3)
                    nc.sync.dma_start(xnat[:], x_d[m:m + P, :])
                    # two half-tiles so mm1 k=0..3 can start after half 0
                    xTh = [work.tile([P, NSLICE], F32R, tag=f"xT{h}", name="xT")
                           for h in range(2)]

                    def xsink(half, ps):
                        nc.vector.tensor_copy(xTh[half][:], ps[:])
                    transpose_split(xnat, xsink)

                    # mm1: e block [128, 2048] across 4 psum banks, fp32r
                    eps = []
                    for ns in range(N_NSL):
                        ep = epsp.tile([P, NSLICE], F32, tag="e", name="eps")
                        for k in range(N_DCHUNK):
                            nc.tensor.matmul(
                                ep[:], xTh[k // 4][:, (k % 4) * P:(k % 4 + 1) * P],
                                yT[ns][:, k * NSLICE:(k + 1) * NSLICE],
                                start=(k == 0), stop=(k == N_DCHUNK - 1),
                            )
                        eps.append(ep)

                    # softmax over the row of 2048
                    rmax4 = stats.tile([P, N_NSL], F32, tag="rmax4")
                    for ns in range(N_NSL):
                        nc.vector.reduce_max(rmax4[:, ns:ns + 1], eps[ns][:], axis=AX)
                    negmax = stats.tile([P, 1], F32, tag="negmax")
                    rmax = stats.tile([P, 1], F32, tag="rmax")
                    nc.vector.reduce_max(rmax[:], rmax4[:], axis=AX)
                    nc.vector.tensor_scalar_mul(negmax[:], rmax[:], -1.0)

                    s4 = stats.tile([P, N_NSL], F32, tag="s4")
                    for ns in range(N_NSL):
                        c0 = ns * NSLICE
                        nc.scalar.activation(
                            pT[mi][:, c0:c0 + NSLICE], eps[ns][:], EXP,
                            bias=negmax[:], accum_out=s4[:, ns:ns + 1],
                        )
                    ssum = stats.tile([P, 1], F32, tag="ssum")
                    nc.vector.reduce_sum(ssum[:], s4[:], axis=AX)
                    rinv = stats.tile([P, 1], F32, tag="rinv")
                    nc.vector.reciprocal(rinv[:], ssum[:])
                    nc.vector.tensor_scalar_mul(xs[mi][:], xnat[:], rinv[:])

            # ---- phase 2: out[nch, dh] = sum_mi p[mi].T @ xs[mi], one flush ----
            with tc.tile_pool(name="ops", bufs=8, space="PSUM") as opsp:
                for nch in range(N_NCHUNK):
                    for dh in range(N_DHALF):
                        ops = opsp.tile([P, NSLICE], F32, tag="o", name="ops")
                        for mi in range(N_MTILES):
                            nc.tensor.matmul(
                                ops[:],
                                pT[mi][:, nch * P:(nch + 1) * P],
                                xs[mi][:, dh * NSLICE:(dh + 1) * NSLICE],
                                start=(mi == 0), stop=(mi == N_MTILES - 1),
                            )
                        dst = out_d[nch * P:(nch + 1) * P,
                                    dh * NSLICE:(dh + 1) * NSLICE]
                        stage = work.tile([P, NSLICE], F32, tag="ostage", bufs=4)
                        nc.vector.tensor_copy(stage[:], ops[:])
                        nc.sync.dma_start(dst, stage[:])

    nc.compile()
    return nc


_NC_CACHE = {}


def _get_nc():
    if "nc" not in _NC_CACHE:
        _NC_CACHE["nc"] = _build_nc()
    return _NC_CACHE["nc"]


def kernel(x: np.ndarray, y: np.ndarray) -> np.ndarray:
    assert x.shape == (B, M, D) and y.shape == (B, N, D)
    nc = _get_nc()
    ident = np.eye(P, dtype=np.float32)
    in_maps = [
        {
            "x": np.ascontiguousarray(x[b], dtype=np.float32),
            "y": np.ascontiguousarray(y[b], dtype=np.float32),
            "ident": ident,
        }
        for b in range(B)
    ]
    res = bass_utils.run_bass_kernel_spmd(nc, in_maps, core_ids=list(range(B)))
    return np.stack([res.results[b]["out"] for b in range(B)], axis=0)


# revision 11
# speedup vs baseline: 1.9204x; 1.0358x over previous
"""InnerAttention kernel for 8 Trainium2 NeuronCores.

Computes, per batch b:
    e = x[b] @ y[b].T          [M, N]
    p = softmax(e, axis=-1)    (over n)
    out[b] = p.T @ x[b]        [N, D]

Sharding: data-parallel over batch (B=8 -> one batch per core). Full inputs in,
full output out; per-core slices are shipped via run_bass_kernel_spmd in_maps.

Per-core algorithm (M=N=2048, D=1024, P=128):
  phase 0: y -> yT (fp32r, d-major, per-n-slice tiles) via PE transposes
  phase 1: per m-tile (16):
    x m-tile -> xT (fp32r, d-major) via PE transposes
    mm1: e[128, 2048] in PSUM, single fp32r pass (1 cycle/row at free=512)
    softmax: DVE row-max over PSUM, ACT exp (bias=-max) -> p bf16 in SBUF,
         accum_out gives row-sum; 1/sum folded into xs = x * (1/s) in bf16
  phase 2: per (n-chunk 128, d-half 512) out tile: accumulate all 16
    p.T @ xs contributions (bf16) in one PSUM bank, stage, single DMA out.
"""

import numpy as np

import concourse.bacc as bacc
import concourse.mybir as mybir
import concourse.tile as tile
from concourse import bass_utils

B, M, N, D = 8, 2048, 2048, 1024
P = 128
NSLICE = 512          # matmul moving free-dim (one PSUM bank of fp32)
N_MTILES = M // P     # 16
N_DCHUNK = D // P     # 8
N_NSL = N // NSLICE   # 4
N_NCHUNK = N // P     # 16
N_DHALF = D // NSLICE  # 2

F32 = mybir.dt.float32
F32R = mybir.dt.float32r
BF16 = mybir.dt.bfloat16
AX = mybir.AxisListType.X
EXP = mybir.ActivationFunctionType.Exp


def _build_nc():
    nc = bacc.Bacc("TRN2", target_bir_lowering=False, debug=False)
    x_d = nc.dram_tensor("x", [M, D], F32R, kind="ExternalInput").ap()
    y_d = nc.dram_tensor("y", [N, D], F32R, kind="ExternalInput").ap()
    ident_d = nc.dram_tensor("ident", [P, P], F32R, kind="ExternalInput").ap()
    out_d = nc.dram_tensor("out", [N, D], F32, kind="ExternalOutput").ap()

    with tile.TileContext(nc) as tc:
        with (
            tc.tile_pool(name="const", bufs=1) as constp,
            tc.tile_pool(name="yT", bufs=1) as yTp,
            tc.tile_pool(name="pP", bufs=1) as pPp,
            tc.tile_pool(name="xsP", bufs=1) as xsPp,
            tc.tile_pool(name="work", bufs=2) as work,
            tc.tile_pool(name="stats", bufs=3) as stats,
        ):
            # identity for PE transposes (fp32r: 1.5 cycles/row vs 2.0 for f32)
            ident = constp.tile([P, P], F32R, tag="id32")
            nc.sync.dma_start(ident[:], ident_d)

            # persistent yT: per n-slice tiles [128 d, 8 chunks x 512 n]
            # fp32r; chunk k occupies columns [k*512, (k+1)*512)
            yT = [yTp.tile([P, N_DCHUNK * NSLICE], F32R, tag=f"yT{s}", name=f"yT{s}")
                  for s in range(N_NSL)]
            # persistent p (bf16) and scaled-x (bf16) for mm2
            pT = [pPp.tile([P, N], BF16, tag=f"p{mi}", name=f"p{mi}")
                  for mi in range(N_MTILES)]
            xs = [xsPp.tile([P, D], BF16, tag=f"xs{mi}", name=f"xs{mi}")
                  for mi in range(N_MTILES)]

            with (
                tc.tile_pool(name="tps", bufs=2, space="PSUM") as tpsp,
                tc.tile_pool(name="eps", bufs=6, space="PSUM") as epsp,
            ):
                def transpose_split(src, sink):
                    """PE-transpose fp32r [128, 1024] in two 4-chunk batches;
                    sink consumes (half, psum [128, 4x128] fp32r)."""
                    for half in range(2):
                        ps = tpsp.tile([P, NSLICE], F32R, tag="tp", name="tp")
                        for j in range(4):
                            k = half * 4 + j
                            nc.tensor.transpose(
                                ps[:, j * P:(j + 1) * P],
                                src[:, k * P:(k + 1) * P],
                                ident[:],
                            )
                        sink(half, ps)

                # x m-tile load + PE-transpose, software-pipelined one tile
                # ahead of mm1 so the DVE psum->xT copies hide under mm1
                # streams of the previous tile.
                xT_of = {}
                xnat_of = {}

                def emit_xpose(mi):
                    xnat = work.tile([P, D], F32R, tag="xnat", bufs=3)
                    nc.sync.dma_start(xnat[:], x_d[mi * P:(mi + 1) * P, :])
                    xTh = [work.tile([P, NSLICE], F32R, tag=f"xT{h}", name="xT")
                           for h in range(2)]

                    def xsink(half, ps):
                        nc.vector.tensor_copy(xTh[half][:], ps[:])
                    transpose_split(xnat, xsink)
                    xT_of[mi] = xTh
                    xnat_of[mi] = xnat

                def emit_mm1_group(mi, ns):
                    """one n-slice of e for m-tile mi -> psum bank (fp32r)"""
                    xTh = xT_of[mi]
                    ep = epsp.tile([P, NSLICE], F32, tag="e", name="eps")
                    for k in range(N_DCHUNK):
                        nc.tensor.matmul(
                            ep[:], xTh[k // 4][:, (k % 4) * P:(k % 4 + 1) * P],
                            yT[ns][:, k * NSLICE:(k + 1) * NSLICE],
                            start=(k == 0), stop=(k == N_DCHUNK - 1),
                        )
                    return ep

                def emit_softmax(mi, eps):
                    rmax4 = stats.tile([P, N_NSL], F32, tag="rmax4")
                    for ns in range(N_NSL):
                        nc.vector.reduce_max(rmax4[:, ns:ns + 1], eps[ns][:], axis=AX)
                    negmax = stats.tile([P, 1], F32, tag="negmax")
                    rmax = stats.tile([P, 1], F32, tag="rmax")
                    nc.vector.reduce_max(rmax[:], rmax4[:], axis=AX)
                    nc.vector.tensor_scalar_mul(negmax[:], rmax[:], -1.0)

                    s4 = stats.tile([P, N_NSL], F32, tag="s4")
                    for ns in range(N_NSL):
                        c0 = ns * NSLICE
                        nc.scalar.activation(
                            pT[mi][:, c0:c0 + NSLICE], eps[ns][:], EXP,
                            bias=negmax[:], accum_out=s4[:, ns:ns + 1],
                        )
                    ssum = stats.tile([P, 1], F32, tag="ssum")
                    nc.vector.reduce_sum(ssum[:], s4[:], axis=AX)
                    rinv = stats.tile([P, 1], F32, tag="rinv")
                    nc.vector.reciprocal(rinv[:], ssum[:])
                    nc.vector.tensor_scalar_mul(xs[mi][:], xnat_of.pop(mi)[:], rinv[:])
                    del xT_of[mi]

                # ---- phase 0 (y -> yT) fused with tile-0 mm1: tile 0's
                # n-slice group ns only needs yT[ns], so it slots into the
                # DMA-gated idle after y-slice ns is transposed. ----
                eps0 = []
                for sl in range(N_NSL):
                    for i in range(4 * sl, 4 * sl + 4):
                        c0 = (i % 4) * P
                        ynat = work.tile([P, D], F32R, tag="ynat", bufs=4)
                        nc.sync.dma_start(ynat[:], y_d[i * P:(i + 1) * P, :])

                        def ysink(half, ps, sl=sl, c0=c0):
                            dst = yT[sl].rearrange("p (k c) -> p k c", c=NSLICE)[
                                :, half * 4:half * 4 + 4, c0:c0 + P]
                            src = ps.rearrange("p (j c) -> p j c", c=P)
                            nc.vector.tensor_copy(dst, src)
                        transpose_split(ynat, ysink)
                    if sl == 0:
                        emit_xpose(0)
                    eps0.append(emit_mm1_group(0, sl))

                # ---- phase 1 steady state ----
                emit_xpose(1)
                emit_softmax(0, eps0)
                for mi in range(1, N_MTILES):
                    eps = []
                    for ns in range(N_NSL):
                        eps.append(emit_mm1_group(mi, ns))
                        if ns == 0 and mi + 1 < N_MTILES:
                            emit_xpose(mi + 1)
                    emit_softmax(mi, eps)

            # ---- phase 2: out[nch, dh] = sum_mi p[mi].T @ xs[mi], one flush ----
            with tc.tile_pool(name="ops", bufs=8, space="PSUM") as opsp:
                for nch in range(N_NCHUNK):
                    for dh in range(N_DHALF):
                        ops = opsp.tile([P, NSLICE], F32, tag="o", name="ops")
                        for mi in range(N_MTILES):
                            nc.tensor.matmul(
                                ops[:],
                                pT[mi][:, nch * P:(nch + 1) * P],
                                xs[mi][:, dh * NSLICE:(dh + 1) * NSLICE],
                                start=(mi == 0), stop=(mi == N_MTILES - 1),
                            )
                        dst = out_d[nch * P:(nch + 1) * P,
                                    dh * NSLICE:(dh + 1) * NSLICE]
                        stage = work.tile([P, NSLICE], F32, tag="ostage", bufs=4)
                        nc.vector.tensor_copy(stage[:], ops[:])
                        nc.sync.dma_start(dst, stage[:])

    nc.compile()
    return nc


_NC_CACHE = {}


def _get_nc():
    if "nc" not in _NC_CACHE:
        _NC_CACHE["nc"] = _build_nc()
    return _NC_CACHE["nc"]


def kernel(x: np.ndarray, y: np.ndarray) -> np.ndarray:
    assert x.shape == (B, M, D) and y.shape == (B, N, D)
    nc = _get_nc()
    ident = np.eye(P, dtype=np.float32)
    in_maps = [
        {
            "x": np.ascontiguousarray(x[b], dtype=np.float32),
            "y": np.ascontiguousarray(y[b], dtype=np.float32),
            "ident": ident,
        }
        for b in range(B)
    ]
    res = bass_utils.run_bass_kernel_spmd(nc, in_maps, core_ids=list(range(B)))
    return np.stack([res.results[b]["out"] for b in range(B)], axis=0)
